# revision 97
# baseline (speedup 1.0000x reference)
"""Trainium2 Bass kernel for 16-head MHA (B=2, S=2048, D=1024, fp32).

Sharding: tensor-parallel over heads across 8 NeuronCores. Core c owns
heads 2c, 2c+1 (model dims c*128..c*128+127): wq/wk/wv column slices,
wo row slice. Each core computes its heads' attention and a rank-128
partial of the output projection in bf16; the host sums the 8 partials
in fp32, undoes the wv host-scale, and adds bo.

Device data flow per core:
  Q/K/V projections are 3-term error-compensated fp8 DoubleRow matmuls
  (x_hi*w_hi + x_hi*w_lo + x_lo*w_hi, each pass contracting 2x128 at
  0.5 cyc/col = 0.75x the bf16 PE cost, ~bf16 accuracy). The host
  splits x and the 64x-prescaled weights into e4m3 hi+lo pairs (the
  prescale clears the e4m3 subnormal range; it cancels exactly via the
  power-of-two exp scale for Q/K and a host-side /64 for V). x arrives
  window-major/chunk-major so each window DMA is 128 contiguous 4KB
  descriptors.
  scores^T tiles [t=128, s=1024] in bf16 -> exp; attn@V is role-
  swapped: exp tiles are the STATIONARY operand, V+ones columns the
  65-wide MOVING operand, so the PE streams 65 cols/chunk and the
  softmax denominator falls out as a free column -> token-major attn
  in PSUM. Normalize with DVE reciprocal + per-partition multiply,
  DMA-transpose into head-dim-major attn_sb, then the bf16 output-
  projection partial.

Engine budget (cost model): PE ~131us busy is the pacer. exp would be
141us on ACT alone, so every 5th tile (plus drain-critical u3-h1
tiles) is computed as a DVE staging multiply (PSUM->SBUF fp32, exp
scale folded in) followed by pow(e, z) on the otherwise-idle GPSIMD
engine - walrus allows TensorTensor pow on Pool, while Activation must
stay on ACT and Pool cannot read PSUM. Busy: ACT ~117us, DVE ~103us,
Pool ~46us, SP ~35us. Score tiles stream against sca/scb PSUM WAR
recycling; K/Q/V chains, attn groups, and output projections fill the
PE gaps stretch by stretch.

Drain: four u3-h1 attn chains pipeline chunk-wise against the last
exps (po banks free after the deferred ops, pa banks after the h0
filler groups). The remaining four groups, PE transposes (identity-
permutation matmuls - DMA-transposes raced the out_proj reads in the
compiled-NEFF path), and the final projection on a 6-bank rotation
(score banks are free after the last exp) compress the tail to the
last out-DMA's completion semaphore.
"""

import os
import sys

import numpy as np

sys.path.insert(0, "/opt/trn_rl_repo")

import ml_dtypes

import concourse.bacc as bacc
import concourse.bass as bass
import concourse.mybir as mybir
import concourse.tile as tile
from concourse.bass_utils import run_bass_kernel_spmd

BF16 = mybir.dt.bfloat16
F32 = mybir.dt.float32
F8 = mybir.dt.float8e4

D = 1024          # model dim
T = 4096          # total tokens (B*S)
S = 2048          # seq len per batch
DC = 128          # per-core head dims (2 heads x 64)
KC = D // 128     # contraction chunks for projections
NCORES = 8
VW = 129          # vp block width: V_h0(64) | ones(1) | V_h1(64)
WSCALE = 64.0     # host pre-scale on wq/wk/wv (clears e4m3 subnormals)
ESCALE = 0.125 / (WSCALE * WSCALE)   # exp scale: 1/sqrt(Hd) / (64q * 64k)

_cache = {"nc": None}
last_exec_time_ns = None


def _build_nc():
    nc = bacc.Bacc("TRN2", target_bir_lowering=False)

    # x and wq/wk/wv are fp8 hi/lo splits (x = xhi + xlo exactly to ~2^-7
    # relative); wq/wk/wv are host-scaled by 64 so their values clear the
    # e4m3 subnormal range, compensated downstream (exp scale / host /64).
    # Projections run as 3-term DoubleRow matmuls (hi*hi + lo*hi + hi*lo),
    # 0.75x the bf16 PE cost with ~bf16 accuracy.
    # host layout: xw[p, w*4096 + q*8 + c] = x[c*128+p, w*512+q] so window
    # DMAs are straight [128, 4096] contiguous copies
    xhi_d = nc.dram_tensor("xhi", [128, T * KC], F8, kind="ExternalInput")
    xlo_d = nc.dram_tensor("xlo", [128, T * KC], F8, kind="ExternalInput")
    # weights pre-reshaped on host to [128, kc*128+dc] so the DMA is one
    # dense [128, 1024] copy (2KB descriptors)
    wq_d = [nc.dram_tensor(f"wq{s}", [128, D], F8, kind="ExternalInput")
            for s in ("h", "l")]
    wk_d = [nc.dram_tensor(f"wk{s}", [128, D], F8, kind="ExternalInput")
            for s in ("h", "l")]
    wv_d = [nc.dram_tensor(f"wv{s}", [128, D], F8, kind="ExternalInput")
            for s in ("h", "l")]
    wo_d = nc.dram_tensor("wo", [DC, D], BF16, kind="ExternalInput")
    ident_d = nc.dram_tensor("ident", [128, 128], BF16, kind="ExternalInput")
    bqk_d = nc.dram_tensor("bqk", [DC, 2], F32, kind="ExternalInput")
    bvb_d = nc.dram_tensor("bvb", [128, DC], F32, kind="ExternalInput")
    out_d = nc.dram_tensor("outp", [D, T], BF16, kind="ExternalOutput")

    with tile.TileContext(nc) as tc:
        _emit(tc, nc, xhi_d, xlo_d, wq_d, wk_d, wv_d, wo_d, bqk_d, bvb_d,
              ident_d, out_d)
    if not nc.is_finalized():
        nc.finalize()
    return nc


def _emit(tc, nc, xhi_d, xlo_d, wq_d, wk_d, wv_d, wo_d, bqk_d, bvb_d,
          ident_d, out_d):
    from contextlib import ExitStack
    stack = ExitStack()
    singles = stack.enter_context(tc.tile_pool(name="singles", bufs=1))

    wq_sb = [singles.tile([128, D], F8, name=f"wq_sb{i}") for i in range(2)]
    wk_sb = [singles.tile([128, D], F8, name=f"wk_sb{i}") for i in range(2)]
    wv_sb = [singles.tile([128, D], F8, name=f"wv_sb{i}") for i in range(2)]
    wo_sb = singles.tile([128, D], BF16, name="wo_sb")
    ident_sb = singles.tile([128, 128], BF16, name="ident_sb")
    bqk_sb = singles.tile([DC, 2], F32, name="bqk_sb")
    bvb_sb = singles.tile([128, DC], F32, name="bvb_sb")
    scr = singles.tile([1, 2], F32, name="scr")

    # warm tile for the PE p-state ramp: memset is the very first DVE
    # instruction so warmup matmuls can start at ~300ns
    wtile = singles.tile([128, 512], BF16, name="wtile")
    nc.vector.memset(wtile, 1.0)

    # preload the ACT exp table while DMAs stream
    nc.vector.memset(scr[:, 0:1], 0.0)
    nc.scalar.activation(scr[:, 1:2], scr[:, 0:1],
                         mybir.ActivationFunctionType.Exp)

    # constant-e tile: Pool computes exp(z) = e ** z via tensor_tensor pow
    # (walrus allows pow on GPSIMD; Activation must stay on ACT). The exp
    # scale is folded into the DVE PSUM->SBUF staging multiply, exactly.
    ec_sb = singles.tile([128, 1024], F32, name="ec_sb")
    nc.gpsimd.memset(ec_sb, 2.718281828459045)

    qt_sb = singles.tile([128, T], BF16, name="qt_sb")   # Q^T head-major
    kt_sb = singles.tile([128, T], BF16, name="kt_sb")   # K^T head-major
    # V token-major; per 128-token block tb: cols [tb*129 + 0..63] = V_h0,
    # [+64] = 1.0 (shared denominator column), [+65..128] = V_h1
    vp_sb = singles.tile([128, 32 * VW], BF16, name="vp_sb")
    nc.vector.memset(
        vp_sb.rearrange("p (b w) -> p b w", w=VW)[:, :, 64:65], 1.0)
    attn_sb = singles.tile([128, T], BF16, name="attn_sb")  # attn^T d-major

    with (
        tc.tile_pool(name="xpool", bufs=1) as xpool,
        tc.tile_pool(name="epool", bufs=1) as epool,
        tc.tile_pool(name="gpool", bufs=4) as gpool,
        tc.tile_pool(name="rpool", bufs=4) as rpool,
        tc.tile_pool(name="obuf", bufs=1) as obpool,
        tc.tile_pool(name="spool", bufs=1) as spool,
        tc.tile_pool(name="ps", bufs=1, space="PSUM") as pps,
    ):
        # window-major, chunk-major x: xj*[w][p, c*512 + q] =
        # x[c*128+p, w*512+q]. The host DRAM layout matches exactly, so a
        # window DMA is 128 contiguous 4KB descriptors (vs 1024 strided
        # 512B ones) -> ~2x cheaper sequencer issue. Inner token dim is
        # stride-1, which the Ldweights ISA requires for the V-projection
        # stationary operand.
        xjh = [
            xpool.tile([128, KC * 512], F8, tag=f"xjh{w}", name=f"xjh{w}")
            for w in range(8)
        ]
        xjl = [
            xpool.tile([128, KC * 512], F8, tag=f"xjl{w}", name=f"xjl{w}")
            for w in range(8)
        ]

        def xview(xt_):
            # [p, c, q] view of the window tile
            return xt_.rearrange("p (c q) -> p c q", q=512)

        def load_window(w, eng, half=None):
            pairs = (((xjh, xhi_d),) if half == "h"
                     else ((xjl, xlo_d),) if half == "l"
                     else ((xjh, xhi_d), (xjl, xlo_d)))
            for xj_, xd_ in pairs:
                eng.dma_start(
                    out=xj_[w],
                    in_=xd_[:, w * 4096:(w + 1) * 4096])

        # critical-path DMA order on SP; lo-halves ride the idle Pool
        # queue in parallel. Non-critical windows (xj4-7) and wo are
        # emitted later as schedule fillers so the tile scheduler cannot
        # hoist them ahead of these.
        nc.sync.dma_start(out=wk_sb[0], in_=wk_d[0][:, :])
        nc.sync.dma_start(out=xjh[0][:, 0:1024], in_=xhi_d[:, 0:1024])
        nc.sync.dma_start(out=wk_sb[1], in_=wk_d[1][:, :])
        nc.sync.dma_start(out=xjh[0][:, 1024:4096], in_=xhi_d[:, 1024:4096])
        nc.sync.dma_start(out=wq_sb[0], in_=wq_d[0][:, :])
        nc.sync.dma_start(out=wq_sb[1], in_=wq_d[1][:, :])
        load_window(0, nc.gpsimd, "l")
        nc.scalar.dma_start(out=bqk_sb, in_=bqk_d[:, :])
        load_window(1, nc.sync, "h")
        load_window(1, nc.gpsimd, "l")
        load_window(2, nc.sync, "h")
        load_window(2, nc.gpsimd, "l")
        load_window(3, nc.sync, "h")
        load_window(3, nc.gpsimd, "l")
        nc.sync.dma_start(out=wv_sb[0], in_=wv_d[0][:, :])
        nc.sync.dma_start(out=wv_sb[1], in_=wv_d[1][:, :])
        nc.scalar.dma_start(out=bvb_sb, in_=bvb_d[:, :])

        units = [(b, sw) for b in range(2) for sw in range(2)]
        exp_tiles = {}

        QK_TAGS = ("pa0", "pa1", "po0", "po1")
        qk_i = [0]

        DRM = mybir.MatmulPerfMode.DoubleRow

        def wpair(wt, c2):
            # stationary chunk pair (2c2, 2c2+1): [p, 2, 128]
            return wt[:, c2 * 256:(c2 + 1) * 256].rearrange(
                "p (two m) -> p two m", two=2)

        def kq_chain(kind, j, tag=None, cols=(0, 512)):
            w_sb, dst, bcol = ((wq_sb, qt_sb, 0) if kind == "q"
                               else (wk_sb, kt_sb, 1))
            if tag is None:
                tag = QK_TAGS[qk_i[0] % 4]
                qk_i[0] += 1
            lo, hi = cols
            ptile = pps.tile([128, 512], F32, tag=tag, name=tag)
            n, nlast = 0, 3 * (KC // 2) - 1
            for xt_, wt_ in ((xjh[j], w_sb[0]), (xjh[j], w_sb[1]),
                             (xjl[j], w_sb[0])):
                xr = xview(xt_)
                for c2 in range(KC // 2):
                    nc.tensor.matmul(
                        ptile[:, 0:hi - lo], wpair(wt_, c2),
                        xr[:, 2 * c2:2 * c2 + 2, lo:hi],
                        start=(n == 0), stop=(n == nlast), perf_mode=DRM)
                    n += 1
            nc.vector.tensor_scalar_add(
                dst[:, j * 512 + lo:j * 512 + hi], ptile[:, 0:hi - lo],
                bqk_sb[:, bcol:bcol + 1])

        def v_block(tb, tag=None):
            if tag is None:
                tag = QK_TAGS[qk_i[0] % 4]
                qk_i[0] += 1
            pv = pps.tile([128, 512], F32, tag=tag, name=tag)
            w, off = tb // 4, (tb % 4) * 128
            n, nlast = 0, 3 * (KC // 2) - 1
            for xt_, wt_ in ((xjh[w], wv_sb[0]), (xjh[w], wv_sb[1]),
                             (xjl[w], wv_sb[0])):
                xr = xview(xt_)
                for c2 in range(KC // 2):
                    nc.tensor.matmul(
                        pv[:, 0:128],
                        xr[:, 2 * c2:2 * c2 + 2, off:off + 128],
                        wpair(wt_, c2),
                        start=(n == 0), stop=(n == nlast), perf_mode=DRM)
                    n += 1
            c0 = tb * VW
            nc.vector.tensor_add(vp_sb[:, c0:c0 + 64], pv[:, 0:64],
                                 bvb_sb[:, 0:64])
            nc.vector.tensor_add(vp_sb[:, c0 + 65:c0 + 129], pv[:, 64:128],
                                 bvb_sb[:, 64:128])

        sc_cnt = [0]
        stg_i = [0]
        full_i = [0]

        def sc_tile(u, tt, h, halves=False):
            b, sw = units[u]
            soff = b * S + sw * 1024
            toff = b * S + tt * 128
            tag = "sca" if (sc_cnt[0] % 2 == 0) else "scb"
            sc_cnt[0] += 1
            ps = pps.tile([128, 1024], F32, tag=tag, name=tag)
            e = epool.tile([128, 1024], BF16, tag=f"e_{tt}_{h}", name="e")
            exp_tiles[(u, tt, h)] = e
            # ACT is the serial exp bottleneck; every 5th tile is computed
            # as DVE staging-multiply + Pool pow instead. In the drain
            # stretch (u3-h1) PE filler is thin, so alternate tiles also
            # offload to keep the ACT cadence off the critical path.
            offload = (not halves
                       and ((full_i[0] % 5 == 2 and not (u == 3 and h == 1))
                            or (u == 3 and h == 1 and tt in (9, 11, 13))))
            if not halves:
                full_i[0] += 1

            def half(sc):
                nc.tensor.matmul(
                    ps[:, sc * 512:(sc + 1) * 512],
                    kt_sb[h * 64:(h + 1) * 64, toff:toff + 128],
                    qt_sb[h * 64:(h + 1) * 64,
                          soff + sc * 512:soff + (sc + 1) * 512],
                    start=True, stop=True)
                if halves:
                    nc.scalar.activation(
                        e[:, sc * 512:(sc + 1) * 512],
                        ps[:, sc * 512:(sc + 1) * 512],
                        mybir.ActivationFunctionType.Exp, scale=ESCALE)

            if halves:
                return half
            half(0)
            half(1)
            if offload:
                stg = spool.tile([128, 1024], F32,
                                 tag=f"stg{stg_i[0] % 3}", name="stg")
                stg_i[0] += 1
                nc.vector.tensor_scalar_mul(stg, ps, ESCALE)
                nc.gpsimd.tensor_tensor(out=e, in0=ec_sb, in1=stg,
                                        op=mybir.AluOpType.pow)
            elif u == 3 and h == 1 and tt == 15:
                # the whole drain hangs off this one exp: split halves so
                # the first 512 columns release ~500ns earlier
                for sc in range(2):
                    nc.scalar.activation(
                        e[:, sc * 512:(sc + 1) * 512],
                        ps[:, sc * 512:(sc + 1) * 512],
                        mybir.ActivationFunctionType.Exp, scale=ESCALE)
            else:
                nc.scalar.activation(
                    e, ps, mybir.ActivationFunctionType.Exp, scale=ESCALE)

        gathers = {}

        def _gather(b, sb):
            key = (b, sb)
            if key not in gathers:
                gathers[key] = gpool.tile([128, 128], BF16,
                                          tag=f"g{sb % 4}", name="g")
            return gathers[key]

        def attn_group(u, h, lb, tag=None, drain=False):
            b, sw = units[u]
            sb = sw * 8 + lb
            if tag is None:
                tag = "pa0" if ((h * 8 + lb) % 2 == 0) else "pa1"
            pa = pps.tile([128, 512], F32, tag=tag, name=tag)
            for tt in range(16):
                c0 = (b * 16 + tt) * VW + h * 64
                nc.tensor.matmul(
                    pa[:, 0:65],
                    exp_tiles[(u, tt, h)][:, lb * 128:(lb + 1) * 128],
                    vp_sb[:, c0:c0 + 65],
                    start=(tt == 0), stop=(tt == 15))
            # h0: cols 0:64 attn, col 64 denom; h1: col 0 denom, 1:65 attn
            dcol, voff = (64, 0) if h == 0 else (0, 1)
            rr = rpool.tile([128, 1], F32, tag=f"rr{(h * 8 + lb) % 4}",
                            name="rr")
            nc.vector.reciprocal(rr, pa[:, dcol:dcol + 1])
            g = _gather(b, sb)
            if drain:
                # drain: ACT (idle after the last exp) does the scale so
                # DVE recip and ACT mul pipeline across engines
                nc.scalar.activation(
                    g[:, h * 64:(h + 1) * 64], pa[:, voff:voff + 64],
                    mybir.ActivationFunctionType.Copy, scale=rr)
            else:
                nc.vector.tensor_scalar_mul(
                    g[:, h * 64:(h + 1) * 64], pa[:, voff:voff + 64], rr)

        def attn_transpose(u, lb, eng=None):
            b, sw = units[u]
            sb = sw * 8 + lb
            g = gathers.pop((b, sb))
            (eng or nc.sync).dma_start_transpose(
                out=attn_sb[:, b * S + sb * 128:b * S + (sb + 1) * 128],
                in_=g)

        ob_tiles = {}

        def out_proj(u, jc):
            out_proj_part(u, jc, range(KC))

        def out_proj_part(u, jc, dts):
            b, sw = units[u]
            soff = b * S + sw * 1024
            for dt in dts:
                if u == 3:
                    # drain: 6-bank rotation (sca/scb are free after the
                    # last exp) and DVE/ACT-alternated copies (ACT is idle
                    # after the last exp; the copies are the serial
                    # element of the tail otherwise)
                    tag = ("sca", "scb", "po0", "po1", "pa0", "pa1")[dt % 6]
                else:
                    tag = "po0" if dt % 2 == 0 else "po1"
                po = pps.tile([128, 512], F32, tag=tag, name=tag)
                nc.tensor.matmul(
                    po, wo_sb[:, dt * 128:(dt + 1) * 128],
                    attn_sb[:, soff + jc * 512:soff + (jc + 1) * 512],
                    start=True, stop=True)
                if jc == 0:
                    ob_tiles[(u, dt)] = obpool.tile(
                        [128, 1024], BF16, tag=f"ob{dt}", name="ob")
                ob = ob_tiles[(u, dt)]
                dst = ob[:, jc * 512:(jc + 1) * 512]
                if u == 3 and dt % 2 == 1:
                    nc.scalar.copy(dst, po)
                else:
                    nc.vector.tensor_copy(dst, po)
                if u == 3:
                    # drain: move each 512-half as soon as it is copied so
                    # the kernel-ending DMA is half-sized
                    eng = nc.gpsimd if dt % 2 == 0 else nc.sync
                    eng.dma_start(
                        out=out_d[dt * 128:(dt + 1) * 128,
                                  soff + jc * 512:soff + (jc + 1) * 512],
                        in_=ob_tiles[(u, dt)][:, jc * 512:(jc + 1) * 512])
                    if jc == 1:
                        ob_tiles.pop((u, dt))
                elif jc == 1:
                    # out DMAs go on SP (cheap issue, end of its queue);
                    # keeping them off Pool leaves the Pool FIFO exp-only
                    # so offloaded exps never queue behind a 1us DMA issue.
                    nc.sync.dma_start(
                        out=out_d[dt * 128:(dt + 1) * 128,
                                  soff:soff + 1024],
                        in_=ob_tiles.pop((u, dt)))

        # ---- emission schedule ----
        # Four stretches of 32 score tiles (one per unit), ACT-paced.
        # Fillers per stretch are levelled to ~19us against the 33us ACT
        # window; attn groups of unit u are front-packed into stretch u+2
        # halves so the shared e-buffers recycle just ahead of ACT.

        def run_stretch(u, h0_fill, h1_fill):
            for tt in range(16):
                for w in h0_fill.get(tt, ()):
                    w()
                sc_tile(u, tt, 0)
            for tt in range(16):
                for w in h1_fill.get(tt, ()):
                    w()
                sc_tile(u, tt, 1)

        def F(fn, *a):
            return lambda: fn(*a)

        # p-state ramp: start the PE immediately on a DVE-memset tile so
        # the clock is ramping while the first DMAs are still in flight;
        # results are never read.
        warm = pps.tile([128, 512], F32, tag="pa0", name="pa0")
        for _ in range(5):
            nc.tensor.matmul(warm, wtile[:, 0:128], wtile[:, 0:512],
                             start=True, stop=True)

        # prologue: a 128-col K partial for token block 0 plus half-tile
        # score/exp ops lets the first exp fire ~5us earlier than waiting
        # for three full 512-col chains.
        ptt0 = pps.tile([128, 512], F32, tag="pa0", name="pa0")
        n = 0
        for xt_, wt_ in ((xjh[0], wk_sb[0]), (xjh[0], wk_sb[1]),
                         (xjl[0], wk_sb[0])):
            xr = xview(xt_)
            for c2 in range(KC // 2):
                nc.tensor.matmul(ptt0[:, 0:128], wpair(wt_, c2),
                                 xr[:, 2 * c2:2 * c2 + 2, 0:128],
                                 start=(n == 0), stop=(n == 11),
                                 perf_mode=DRM)
                n += 1
        nc.vector.tensor_scalar_add(kt_sb[:, 0:128], ptt0[:, 0:128],
                                    bqk_sb[:, 1:2])
        kq_chain("q", 0, "pa1")
        h00 = sc_tile(0, 0, 0, halves=True)
        h01 = sc_tile(0, 0, 1, halves=True)
        h00(0)
        h01(0)
        kq_chain("q", 1, "po0")
        h00(1)
        h01(1)

        # K j0 chain, skipping the already-computed token block 0
        kq_chain("k", 0, "po1", cols=(128, 512))

        # stretch 1 (u0): rest of b0 K/Q chains + all b0 V blocks,
        # thinned to one chain per ~3 score tiles so ACT is never starved
        s1_h0 = {1: [F(kq_chain, "k", 1)], 4: [F(kq_chain, "k", 2)],
                 7: [F(kq_chain, "k", 3)], 10: [F(kq_chain, "q", 2)],
                 13: [F(kq_chain, "q", 3)], 15: [F(v_block, 0)]}
        # V blocks 1-15 packed two-per-tile early so the spilled attn(0,0)
        # groups at the tail see a fully-written vp
        s1_h1 = {}
        for i in range(1, 15):
            s1_h1.setdefault((i - 1) // 2, []).append(F(v_block, i))
        s1_h1.setdefault(7, []).append(F(v_block, 15))
        s1_h1.setdefault(1, []).append(F(load_window, 4, nc.gpsimd))
        s1_h1.setdefault(5, []).append(
            lambda: nc.gpsimd.dma_start(out=wo_sb, in_=wo_d[:, :]))
        s1_h1.setdefault(6, []).append(
            lambda: nc.gpsimd.dma_start(out=ident_sb, in_=ident_d[:, :]))
        s1_h1.setdefault(8, []).append(F(load_window, 5, nc.gpsimd))
        s1_h1.setdefault(14, []).append(F(load_window, 6, nc.gpsimd))
        for i in range(8):
            s1_h1.setdefault(8 + i, []).append(F(attn_group, 0, 0, i))
        for tt in range(1, 16):
            for w in s1_h0.get(tt, ()):
                w()
            sc_tile(0, tt, 0)
        for tt in range(16):
            for w in s1_h1.get(tt, ()):
                w()
            sc_tile(0, tt, 1)

        # stretch 2 (u1): attn(u0) + transposes + op(u0) + b1 V blocks
        s2_h0 = {}
        s2_h0.setdefault(0, []).append(F(load_window, 7, nc.gpsimd))
        s2_h0.setdefault(2, []).append(F(kq_chain, "k", 4, "po0"))
        s2_h0.setdefault(5, []).append(F(kq_chain, "q", 4, "po1"))
        s2_h0.setdefault(8, []).append(F(kq_chain, "q", 5, "po0"))
        for i in range(8):
            s2_h0.setdefault(8 + i, []).append(
                F(v_block, 16 + i, "po0" if i % 2 == 0 else "po1"))
        s2_h1 = {}
        for lb in range(8):
            s2_h1.setdefault(lb, []).append(F(attn_group, 0, 1, lb))
            s2_h1.setdefault(lb, []).append(F(attn_transpose, 0, lb))

        s2_h1.setdefault(9, []).append(F(out_proj, 0, 0))
        for i in range(8):
            s2_h1.setdefault(8 + i, []).append(F(attn_group, 1, 0, i))
        run_stretch(1, s2_h0, s2_h1)

        # stretch 3 (u2): attn(u1) all before the sca-WAR-stalled first
        # score tile, then the b1 K chains fill the stall window
        s3_h0 = {}
        s3_h0.setdefault(0, []).append(F(kq_chain, "k", 5, "po1"))
        s3_h0.setdefault(1, []).append(F(kq_chain, "k", 6, "po0"))
        s3_h0.setdefault(3, []).append(F(kq_chain, "k", 7, "po1"))
        s3_h0.setdefault(6, []).append(F(out_proj, 0, 1))
        for i in range(8):
            s3_h0.setdefault(8 + i, []).append(
                F(v_block, 24 + i, "pa0" if i % 2 == 0 else "pa1"))
        s3_h1 = {0: [F(kq_chain, "q", 6, "po0")],
                 1: [F(kq_chain, "q", 7, "po1")]}
        for lb in range(8):
            s3_h1.setdefault(lb, []).append(F(attn_group, 1, 1, lb))
            s3_h1.setdefault(lb, []).append(F(attn_transpose, 1, lb))
        for i in range(8):
            s3_h1.setdefault(8 + i, []).append(F(attn_group, 2, 0, i))
        run_stretch(2, s3_h0, s3_h1)

        # stretch 4 (u3): attn(u2) + transposes + deferred op(u1), op(u2)
        s4_h0 = {}
        s4_h0.setdefault(2, []).append(F(out_proj, 1, 0))
        for lb in range(8):
            s4_h0.setdefault(4 + lb, []).append(F(attn_group, 2, 1, lb))
            s4_h0.setdefault(4 + lb, []).append(F(attn_transpose, 2, lb))
        s4_h1 = {}
        s4_h1.setdefault(1, []).append(F(out_proj, 1, 1))
        s4_h1.setdefault(4, []).append(F(out_proj, 2, 0))
        s4_h1.setdefault(7, []).append(F(out_proj, 2, 1))
        for lb in range(8):
            s4_h1.setdefault(lb, []).append(F(attn_group, 3, 0, lb))

        # four u3-h1 attn chains pipelined chunk-wise against the last
        # exps: lb0/1 on the po banks (free after the ops above drain),
        # lb2/3 on the pa banks (free after the h0 filler groups)
        pipe = {}
        PIPE_TAGS = ("po0", "po1", "pa0", "pa1")

        def pipe_link(ci, tt):
            h, lb = 1, ci
            b = 1
            tag = PIPE_TAGS[ci]
            if ci not in pipe:
                pipe[ci] = pps.tile([128, 512], F32, tag=tag, name=tag)
            c0 = (b * 16 + tt) * VW + h * 64
            nc.tensor.matmul(
                pipe[ci][:, 0:65],
                exp_tiles[(3, tt, h)][:, lb * 128:(lb + 1) * 128],
                vp_sb[:, c0:c0 + 65],
                start=(tt == 0), stop=(tt == 15))

        for tt in range(16):
            for w in s4_h0.get(tt, ()):
                w()
            sc_tile(3, tt, 0)
        for tt in range(16):
            for w in s4_h1.get(tt, ()):
                w()
            sc_tile(3, tt, 1)
            if tt >= 10:
                for ci in (0, 1):
                    for k in ((tt - 10) * 2, (tt - 10) * 2 + 1):
                        if k <= 15:
                            pipe_link(ci, k)
            if tt >= 12:
                for ci in (2, 3):
                    for k in range((tt - 12) * 3, (tt - 12) * 3 + 3):
                        if k <= 11:
                            pipe_link(ci, k)
        for ci in (0, 1):
            for k in (12, 13, 14, 15):
                pipe_link(ci, k)
        for ci in (2, 3):
            for k in (12, 13, 14, 15):
                pipe_link(ci, k)

        def finish_pipe(ci):
            pa = pipe[ci]
            lb = ci
            b, sb = 1, 8 + lb
            rr = rpool.tile([128, 1], F32, tag=f"rr{lb % 4}", name="rr")
            nc.vector.reciprocal(rr, pa[:, 0:1])
            g = _gather(b, sb)
            if ci % 2 == 1:
                # ACT is idle after the last exp; splitting the four
                # normalizes across DVE/ACT halves the serial latency
                nc.scalar.activation(
                    g[:, 64:128], pa[:, 1:65],
                    mybir.ActivationFunctionType.Copy, scale=rr)
            else:
                nc.vector.tensor_scalar_mul(g[:, 64:128], pa[:, 1:65], rr)

        # drain u3: the two pipelined chains finish at the last exp; the
        # remaining six groups rotate over four banks, with the jc=0
        # output projection interleaved as soon as its attn columns
        # (sb 8..11 = lb 0..3) are transposed, so the tail after the
        # last group is just the jc=1 projection.
        finish_pipe(0)
        finish_pipe(1)
        finish_pipe(2)
        finish_pipe(3)
        DR = ("pa0", "pa1", "po0", "po1")

        def pe_transpose(lb, use_act):
            # PE transpose keeps the transpose->out_proj ordering inside
            # the PE FIFO (the drain DMA-transpose raced the out_proj
            # reads in the compiled-NEFF path)
            sb = 8 + lb
            g = gathers.pop((1, sb))
            pt = pps.tile([128, 128], BF16, tag=DR[lb % 4], name="pt")
            nc.tensor.matmul(pt, g, ident_sb, is_transpose=True)
            dst = attn_sb[:, S + sb * 128:S + (sb + 1) * 128]
            if use_act:
                nc.scalar.copy(dst, pt)
            else:
                nc.vector.tensor_copy(dst, pt)

        pe_transpose(0, False)
        pe_transpose(1, True)
        pe_transpose(2, False)
        pe_transpose(3, True)
        out_proj_part(3, 0, range(KC))
        attn_group(3, 1, 4, DR[0])
        attn_group(3, 1, 5, DR[1])
        attn_group(3, 1, 6, DR[2], drain=True)
        attn_group(3, 1, 7, DR[3], drain=True)
        pe_transpose(4, False)
        pe_transpose(5, False)
        pe_transpose(6, False)
        pe_transpose(7, False)
        out_proj_part(3, 1, range(KC))

    stack.close()


def kernel(x, wq, bq, wk, bk, wv, bv, wo, bo):
    global last_exec_time_ns
    bf16 = ml_dtypes.bfloat16
    f8 = ml_dtypes.float8_e4m3
    x = np.asarray(x, dtype=np.float32)
    xt = x.reshape(T, D).T  # [D, T] f32
    # window-major chunk-major layout (see _build_nc)
    xw = np.ascontiguousarray(
        xt.reshape(KC, 128, 8, 512).transpose(1, 2, 0, 3)
        .reshape(128, T * KC))

    def split8(a):
        hi = a.astype(f8)
        lo = (a - hi.astype(np.float32)).astype(f8)
        return hi, lo

    xhi, xlo = split8(xw)

    def preshape(w):
        # [D, DC] -> [128, KC*128]: wsb[p, c*128+m] = w[c*128+p, m]
        return np.ascontiguousarray(
            np.asarray(w, np.float32).reshape(KC, 128, DC)
            .transpose(1, 0, 2).reshape(128, KC * DC))

    wq = np.asarray(wq, np.float32)
    wk = np.asarray(wk, np.float32)
    wv = np.asarray(wv, np.float32)
    bq = np.asarray(bq, np.float32)
    bk = np.asarray(bk, np.float32)
    bv = np.asarray(bv, np.float32)
    wo = np.asarray(wo, np.float32)

    in_maps = []
    for c in range(NCORES):
        sl = slice(c * DC, (c + 1) * DC)
        bvb = np.broadcast_to(
            (bv[sl] * WSCALE)[None, :], (128, DC))
        wqh, wql = split8(preshape(wq[:, sl] * WSCALE))
        wkh, wkl = split8(preshape(wk[:, sl] * WSCALE))
        wvh, wvl = split8(preshape(wv[:, sl] * WSCALE))
        in_maps.append({
            "xhi": xhi,
            "xlo": xlo,
            "wqh": wqh, "wql": wql,
            "wkh": wkh, "wkl": wkl,
            "wvh": wvh, "wvl": wvl,
            "wo": np.ascontiguousarray(wo[sl, :]).astype(bf16),
            "ident": np.eye(128, dtype=bf16),
            "bqk": np.ascontiguousarray(
                np.stack([bq[sl], bk[sl]], axis=1) * WSCALE),
            "bvb": np.ascontiguousarray(bvb, dtype=np.float32),
        })

    if _cache["nc"] is None:
        _cache["nc"] = _build_nc()
    nc = _cache["nc"]

    trace = os.environ.get("BASS_KERNEL_TRACE", "0") == "1"
    try:
        res = run_bass_kernel_spmd(nc, in_maps, core_ids=list(range(NCORES)),
                                   trace=trace)
    except ModuleNotFoundError:
        res = run_bass_kernel_spmd(nc, in_maps, core_ids=list(range(NCORES)),
                                   trace=False)
    last_exec_time_ns = res.exec_time_ns

    partial = np.zeros((D, T), dtype=np.float32)
    for r in res.results:
        partial += r["outp"].astype(np.float32)
    # V carries the host-side WSCALE on wv; undo it after the partial sum
    out = partial.T * (1.0 / WSCALE) + np.asarray(bo, dtype=np.float32)
    return out.reshape(2, S, D).astype(np.float32)

